# revision 17
# baseline (speedup 1.0000x reference)
"""MoE gate (softmax -> top-8 -> renorm + aux-loss stats) on 8 trn2 cores.

Sharding: data-parallel over the flattened token dim (2048 tokens/core).
Each core receives its x-shard pre-transposed to [D, T] so the contraction
dim (D) lies on SBUF partitions, plus the replicated gate weight transposed
to [D, E].

Device kernel (per core):
  - 16 d-chunk matmuls accumulate logits [128 tok, 64 exp] in PSUM
  - ScalarE: exp + row-sum (accum_out) in one instruction
  - VectorE: reciprocal, max (top-8 desc), max_index (exact argmax-8),
    top-8 renorm (sum/recip/mul)
  - pi partial: PE matmul with the reciprocal column stationary:
    [1,64] += r.T @ exp  == column-sums of softmax scores
Host: gathers per-core outputs, computes fi from an index histogram and
the scalar aux loss (64-element math).
"""

import os
import sys

import numpy as np

for _p in ("/opt/trn_rl_repo", "/root/.axon_site/_ro/trn_rl_repo"):
    if os.path.isdir(_p) and _p not in sys.path:
        sys.path.insert(0, _p)

import concourse.bacc as bacc
import concourse.mybir as mybir
import concourse.tile as tile
from concourse import masks
from concourse.bass_utils import run_bass_kernel_spmd

B, S, D = 4, 4096, 2048
E, K = 64, 8
NCORES = 8
N = B * S
T = N // NCORES            # 2048 tokens per core
TB = 512                   # tokens per DMA block
NB = T // TB               # 4 blocks
TT = TB // 128             # 4 token-tiles per block
DC = D // 128              # 16 contraction chunks
AUX_ALPHA = np.float32(0.01)

_cache = {}


def _build_nc():
    f32 = mybir.dt.float32
    u32 = mybir.dt.uint32
    nc = bacc.Bacc(None)
    xt = nc.dram_tensor("xt", [D, T], f32, kind="ExternalInput")
    # weight pre-arranged on host to the SBUF layout: [128, DC*E], where
    # column block c holds wT[c*128:(c+1)*128, :] for partition rows 0..127
    wt = nc.dram_tensor("wt", [128, DC * E], f32, kind="ExternalInput")
    w8o = nc.dram_tensor("w8o", [T, K], f32, kind="ExternalOutput")
    i8o = nc.dram_tensor("i8o", [T, K], u32, kind="ExternalOutput")
    pio = nc.dram_tensor("pio", [1, E], f32, kind="ExternalOutput")

    with tile.TileContext(nc) as tc:
        with (
            tc.tile_pool(name="wpool", bufs=1) as wpool,
            tc.tile_pool(name="xpool", bufs=44) as xpool,
            tc.tile_pool(name="lpool", bufs=2) as lpool,
            tc.tile_pool(name="epool", bufs=4) as epool,
            tc.tile_pool(name="opool", bufs=2) as opool,
            tc.tile_pool(name="small", bufs=4) as spool,
            tc.tile_pool(name="acc", bufs=2) as apool,
            tc.tile_pool(name="lgpsum", bufs=2, space="PSUM") as lgpool,
            tc.tile_pool(name="trpsum", bufs=3, space="PSUM") as trpool,
            tc.tile_pool(name="pipsum", bufs=1, space="PSUM") as pipool,
        ):
            # Replicated gate weight, already in SBUF layout on the host:
            # single fully-contiguous DMA on the scalar HWDGE ring so the x
            # stream on the sync ring is not head-of-line blocked.
            wsb = wpool.tile([128, DC * E], f32)
            nc.scalar.dma_start(wsb[:], wt[:])
            ident = wpool.tile([64, 64], f32)
            masks.make_identity(nc, ident[:])
            ones = wpool.tile([128, 1], f32)
            nc.vector.memset(ones[:], 1.0)

            # pi accumulates in PSUM across blocks: pp += ones.T @ sacc_b
            pp = pipool.tile([1, E], f32)

            def epilogue(b, lgs):
                # Per 128-token tile: PE transpose back to [tok, exp],
                # exp+rowsum and score scaling on ACT, top-8 + renorm on DVE.
                wstage = opool.tile([128, TT * K], f32, tag="wstage")
                istage = opool.tile([128, TT * K], u32, tag="istage")
                sacc = apool.tile([128, E], f32, tag="sacc")
                for t in range(TT):
                    tr = trpool.tile([128, E], f32, tag="tr")
                    nc.tensor.transpose(
                        tr[:], lgs[:, t * 128:(t + 1) * 128], ident[:])
                    ex = epool.tile([128, E], f32, tag="ex")
                    esum = spool.tile([128, 1], f32, tag="esum")
                    nc.scalar.activation(
                        ex[:], tr[:], mybir.ActivationFunctionType.Exp,
                        accum_out=esum[:])
                    r = spool.tile([128, 1], f32, tag="r")
                    nc.vector.reciprocal(r[:], esum[:])
                    e8 = spool.tile([128, 8], f32, tag="e8")
                    nc.vector.max(e8[:], ex[:])
                    nc.vector.max_index(istage[:, t * K:(t + 1) * K], e8[:], ex[:])
                    s8 = spool.tile([128, 1], f32, tag="s8")
                    nc.vector.reduce_sum(s8[:], e8[:], axis=mybir.AxisListType.X)
                    rs = spool.tile([128, 1], f32, tag="rs")
                    nc.vector.reciprocal(rs[:], s8[:])
                    nc.vector.tensor_scalar_mul(
                        wstage[:, t * K:(t + 1) * K], e8[:], rs[:])
                    # scores tile s = ex * r on ACT (Copy with per-row scale)
                    s = epool.tile([128, E], f32, tag="s")
                    nc.scalar.activation(
                        s[:], ex[:], mybir.ActivationFunctionType.Copy,
                        scale=r[:])
                    if t == 0:
                        nc.vector.tensor_copy(sacc[:], s[:])
                    else:
                        nc.vector.tensor_add(sacc[:], sacc[:], s[:])
                nc.tensor.matmul(pp[:], ones[:], sacc[:],
                                 start=(b == 0), stop=(b == NB - 1),
                                 skip_group_check=True)
                nc.scalar.dma_start(
                    w8o[b * TB:(b + 1) * TB, :].rearrange(
                        "(t p) k -> p t k", p=128),
                    wstage[:].rearrange("p (t k) -> p t k", t=TT))
                nc.scalar.dma_start(
                    i8o[b * TB:(b + 1) * TB, :].rearrange(
                        "(t p) k -> p t k", p=128),
                    istage[:].rearrange("p (t k) -> p t k", t=TT))

            # Software-pipelined: block b's epilogue is emitted after block
            # b+1's logits matmuls, so PE never stalls on the ACT/DVE chain.
            pending = None
            for b in range(NB):
                xcs = []
                for c in range(DC):
                    xc = xpool.tile([128, TB], f32, tag="xc")
                    nc.sync.dma_start(
                        xc[:], xt[c * 128:(c + 1) * 128, b * TB:(b + 1) * TB])
                    xcs.append(xc)
                # logits.T for the whole block: [64 exp, 512 tok], computed as
                # two concurrent column-group matmul streams (the 64-column
                # stationary only fills half the PE array): even chunks
                # accumulate into psum rows 0:64, odd chunks into 64:128.
                lg = lgpool.tile([128, TB], f32)
                for ci in range(0, DC, 2):
                    nc.tensor.matmul(
                        lg[0:64, :],
                        wsb[:, ci * E:(ci + 1) * E],
                        xcs[ci][:],
                        start=(ci == 0),
                        stop=(ci == DC - 2),
                        tile_position=(0, 0),
                        skip_group_check=True,
                    )
                    nc.tensor.matmul(
                        lg[64:128, :],
                        wsb[:, (ci + 1) * E:(ci + 2) * E],
                        xcs[ci + 1][:],
                        start=(ci == 0),
                        stop=(ci == DC - 2),
                        tile_position=(0, 64),
                        skip_group_check=True,
                    )
                lgt = lpool.tile([64, TB], f32, tag="lgt")
                nc.scalar.copy(lgt[:], lg[64:128, :])
                lgs = lpool.tile([64, TB], f32, tag="lgs")
                nc.vector.tensor_add(lgs[:], lg[0:64, :], lgt[:])
                if pending is not None:
                    epilogue(*pending)
                pending = (b, lgs)
            epilogue(*pending)
            pi_sb = spool.tile([1, E], f32, tag="pisb")
            nc.vector.tensor_copy(pi_sb[:], pp[:])
            nc.sync.dma_start(pio[:], pi_sb[:])
    nc.compile()
    return nc


def _get_nc():
    if "nc" not in _cache:
        _cache["nc"] = _build_nc()
    return _cache["nc"]


def _make_in_maps(x, weight):
    xf = np.ascontiguousarray(
        np.asarray(x, dtype=np.float32).reshape(N, D))
    wT = np.asarray(weight, dtype=np.float32).T          # [D, E]
    # SBUF layout: [128, DC*E], column block c = wT[c*128:(c+1)*128, :]
    wsb = np.ascontiguousarray(
        wT.reshape(DC, 128, E).transpose(1, 0, 2).reshape(128, DC * E))
    in_maps = []
    for cid in range(NCORES):
        xTs = np.ascontiguousarray(xf[cid * T:(cid + 1) * T].T)
        in_maps.append({"xt": xTs, "wt": wsb})
    return in_maps


def _run_device(x, weight, trace=False, **kw):
    nc = _get_nc()
    in_maps = _make_in_maps(x, weight)
    return run_bass_kernel_spmd(nc, in_maps, list(range(NCORES)), trace=trace, **kw)


def _assemble(results):
    w8 = np.concatenate([results[c]["w8o"] for c in range(NCORES)], axis=0)
    i8 = np.concatenate([results[c]["i8o"] for c in range(NCORES)],
                        axis=0).astype(np.int32)
    pi = np.sum(np.stack([results[c]["pio"][0] for c in range(NCORES)]),
                axis=0, dtype=np.float32) / np.float32(N)
    counts = np.bincount(i8.reshape(-1), minlength=E).astype(np.float32)
    fi = counts / np.float32(N * K) * np.float32(E)
    aux = np.float32(np.sum(pi * fi, dtype=np.float32) * AUX_ALPHA)
    return w8, i8, aux


def kernel(x, weight):
    res = _run_device(x, weight, trace=False)
    return _assemble(res.results)


# revision 21
# speedup vs baseline: 1.0334x; 1.0334x over previous
"""MoE gate (softmax -> top-8 -> renorm + aux-loss stats) on 8 trn2 cores.

Sharding: data-parallel over the flattened token dim (2048 tokens/core).
Each core receives its x-shard pre-transposed to [D, T] so the contraction
dim (D) lies on SBUF partitions, plus the replicated gate weight transposed
to [D, E].

Device kernel (per core):
  - 16 d-chunk matmuls accumulate logits [128 tok, 64 exp] in PSUM
  - ScalarE: exp + row-sum (accum_out) in one instruction
  - VectorE: reciprocal, max (top-8 desc), max_index (exact argmax-8),
    top-8 renorm (sum/recip/mul)
  - pi partial: PE matmul with the reciprocal column stationary:
    [1,64] += r.T @ exp  == column-sums of softmax scores
Host: gathers per-core outputs, computes fi from an index histogram and
the scalar aux loss (64-element math).
"""

import os
import sys

import numpy as np

for _p in ("/opt/trn_rl_repo", "/root/.axon_site/_ro/trn_rl_repo"):
    if os.path.isdir(_p) and _p not in sys.path:
        sys.path.insert(0, _p)

import concourse.bacc as bacc
import concourse.mybir as mybir
import concourse.tile as tile
from concourse import masks
from concourse.bass_utils import run_bass_kernel_spmd

B, S, D = 4, 4096, 2048
E, K = 64, 8
NCORES = 8
N = B * S
T = N // NCORES            # 2048 tokens per core
TB = 512                   # tokens per DMA block
NB = T // TB               # 4 blocks
TT = TB // 128             # 4 token-tiles per block
DC = D // 128              # 16 contraction chunks
AUX_ALPHA = np.float32(0.01)

_cache = {}


def _build_nc():
    f32 = mybir.dt.float32
    u32 = mybir.dt.uint32
    nc = bacc.Bacc(None)
    xt = nc.dram_tensor("xt", [D, T], f32, kind="ExternalInput")
    # weight pre-arranged on host to the SBUF layout: [128, DC*E], where
    # column block c holds wT[c*128:(c+1)*128, :] for partition rows 0..127
    wt = nc.dram_tensor("wt", [128, DC * E], f32, kind="ExternalInput")
    w8o = nc.dram_tensor("w8o", [T, K], f32, kind="ExternalOutput")
    i8o = nc.dram_tensor("i8o", [T, K], u32, kind="ExternalOutput")
    pio = nc.dram_tensor("pio", [1, E], f32, kind="ExternalOutput")

    with tile.TileContext(nc) as tc:
        with (
            tc.tile_pool(name="wpool", bufs=1) as wpool,
            tc.tile_pool(name="xpool", bufs=32) as xpool,
            tc.tile_pool(name="lpool", bufs=2) as lpool,
            tc.tile_pool(name="epool", bufs=4) as epool,
            tc.tile_pool(name="opool", bufs=2) as opool,
            tc.tile_pool(name="small", bufs=4) as spool,
            tc.tile_pool(name="acc", bufs=2) as apool,
            tc.tile_pool(name="lgpsum", bufs=2, space="PSUM") as lgpool,
            tc.tile_pool(name="trpsum", bufs=3, space="PSUM") as trpool,
            tc.tile_pool(name="pipsum", bufs=1, space="PSUM") as pipool,
            tc.tile_pool(name="wupsum", bufs=1, space="PSUM") as wupool,
        ):
            # Replicated gate weight, already in SBUF layout on the host:
            # single fully-contiguous DMA, first in the sync-ring queue so it
            # completes before the first matmul needs it.
            wsb = wpool.tile([128, DC * E], f32)
            nc.sync.dma_start(wsb[:], wt[:])
            ident = wpool.tile([64, 64], f32)
            masks.make_identity(nc, ident[:])
            ones = wpool.tile([128, 1], f32)
            nc.vector.memset(ones[:], 1.0)

            # HAM warm-up: a burst of throwaway bf16 matmuls during the
            # initial DMA fill window pushes the PE clock gate to 8/8
            # before the real (fp32) matmul stream begins.
            bf16 = mybir.dt.bfloat16
            wu_w = wpool.tile([128, 128], bf16)
            nc.vector.memset(wu_w[:], 0.0)
            wu_x = wpool.tile([128, TB], bf16)
            nc.vector.memset(wu_x[:], 0.0)
            wu_ps = wupool.tile([128, TB], f32)
            for _ in range(16):
                nc.tensor.matmul(wu_ps[:], wu_w[:], wu_x[:],
                                 start=True, stop=True)

            # pi accumulates in PSUM across blocks: pp += ones.T @ sacc_b
            pp = pipool.tile([1, E], f32)

            def epilogue(b, lgs):
                # Per 128-token tile: PE transpose back to [tok, exp],
                # exp+rowsum and score scaling on ACT, top-8 + renorm on DVE.
                wstage = opool.tile([128, TT * K], f32, tag="wstage")
                istage = opool.tile([128, TT * K], u32, tag="istage")
                sacc = apool.tile([128, E], f32, tag="sacc")
                for t in range(TT):
                    tr = trpool.tile([128, E], f32, tag="tr")
                    nc.tensor.transpose(
                        tr[:], lgs[:, t * 128:(t + 1) * 128], ident[:])
                    ex = epool.tile([128, E], f32, tag="ex")
                    esum = spool.tile([128, 1], f32, tag="esum")
                    nc.scalar.activation(
                        ex[:], tr[:], mybir.ActivationFunctionType.Exp,
                        accum_out=esum[:])
                    r = spool.tile([128, 1], f32, tag="r")
                    nc.vector.reciprocal(r[:], esum[:])
                    e8 = spool.tile([128, 8], f32, tag="e8")
                    nc.vector.max(e8[:], ex[:])
                    nc.vector.max_index(istage[:, t * K:(t + 1) * K], e8[:], ex[:])
                    s8 = spool.tile([128, 1], f32, tag="s8")
                    nc.vector.reduce_sum(s8[:], e8[:], axis=mybir.AxisListType.X)
                    rs = spool.tile([128, 1], f32, tag="rs")
                    nc.vector.reciprocal(rs[:], s8[:])
                    nc.vector.tensor_scalar_mul(
                        wstage[:, t * K:(t + 1) * K], e8[:], rs[:])
                    # scores tile s = ex * r on ACT (Copy with per-row scale)
                    s = epool.tile([128, E], f32, tag="s")
                    nc.scalar.activation(
                        s[:], ex[:], mybir.ActivationFunctionType.Copy,
                        scale=r[:])
                    if t == 0:
                        nc.vector.tensor_copy(sacc[:], s[:])
                    else:
                        nc.vector.tensor_add(sacc[:], sacc[:], s[:])
                nc.tensor.matmul(pp[:], ones[:], sacc[:],
                                 start=(b == 0), stop=(b == NB - 1),
                                 skip_group_check=True)
                nc.scalar.dma_start(
                    w8o[b * TB:(b + 1) * TB, :].rearrange(
                        "(t p) k -> p t k", p=128),
                    wstage[:].rearrange("p (t k) -> p t k", t=TT))
                nc.scalar.dma_start(
                    i8o[b * TB:(b + 1) * TB, :].rearrange(
                        "(t p) k -> p t k", p=128),
                    istage[:].rearrange("p (t k) -> p t k", t=TT))

            # Software-pipelined: block b's epilogue is emitted after block
            # b+1's logits matmuls, so PE never stalls on the ACT/DVE chain.
            pending = None
            xcs = None
            for b in range(NB):
                if b % 2 == 0:
                    # one [128, 2*TB] DMA per chunk covers two blocks —
                    # halves the DMA count, keeps 4KB-contiguous rows
                    xcs = []
                    for c in range(DC):
                        xc = xpool.tile([128, 2 * TB], f32, tag="xc")
                        nc.sync.dma_start(
                            xc[:],
                            xt[c * 128:(c + 1) * 128, b * TB:(b + 2) * TB])
                        xcs.append(xc)
                half = (b % 2) * TB
                # logits.T for the whole block: [64 exp, 512 tok], computed as
                # two concurrent column-group matmul streams (the 64-column
                # stationary only fills half the PE array): even chunks
                # accumulate into psum rows 0:64, odd chunks into 64:128.
                lg = lgpool.tile([128, TB], f32)
                for ci in range(0, DC, 2):
                    nc.tensor.matmul(
                        lg[0:64, :],
                        wsb[:, ci * E:(ci + 1) * E],
                        xcs[ci][:, half:half + TB],
                        start=(ci == 0),
                        stop=(ci == DC - 2),
                        tile_position=(0, 0),
                        skip_group_check=True,
                    )
                    nc.tensor.matmul(
                        lg[64:128, :],
                        wsb[:, (ci + 1) * E:(ci + 2) * E],
                        xcs[ci + 1][:, half:half + TB],
                        start=(ci == 0),
                        stop=(ci == DC - 2),
                        tile_position=(0, 64),
                        skip_group_check=True,
                    )
                lgt = lpool.tile([64, TB], f32, tag="lgt")
                nc.scalar.copy(lgt[:], lg[64:128, :])
                lgs = lpool.tile([64, TB], f32, tag="lgs")
                nc.vector.tensor_add(lgs[:], lg[0:64, :], lgt[:])
                if pending is not None:
                    epilogue(*pending)
                pending = (b, lgs)
            epilogue(*pending)
            pi_sb = spool.tile([1, E], f32, tag="pisb")
            nc.vector.tensor_copy(pi_sb[:], pp[:])
            nc.sync.dma_start(pio[:], pi_sb[:])
    nc.compile()
    return nc


def _get_nc():
    if "nc" not in _cache:
        _cache["nc"] = _build_nc()
    return _cache["nc"]


def _make_in_maps(x, weight):
    xf = np.ascontiguousarray(
        np.asarray(x, dtype=np.float32).reshape(N, D))
    wT = np.asarray(weight, dtype=np.float32).T          # [D, E]
    # SBUF layout: [128, DC*E], column block c = wT[c*128:(c+1)*128, :]
    wsb = np.ascontiguousarray(
        wT.reshape(DC, 128, E).transpose(1, 0, 2).reshape(128, DC * E))
    in_maps = []
    for cid in range(NCORES):
        xTs = np.ascontiguousarray(xf[cid * T:(cid + 1) * T].T)
        in_maps.append({"xt": xTs, "wt": wsb})
    return in_maps


def _run_device(x, weight, trace=False, **kw):
    nc = _get_nc()
    in_maps = _make_in_maps(x, weight)
    return run_bass_kernel_spmd(nc, in_maps, list(range(NCORES)), trace=trace, **kw)


def _assemble(results):
    w8 = np.concatenate([results[c]["w8o"] for c in range(NCORES)], axis=0)
    i8 = np.concatenate([results[c]["i8o"] for c in range(NCORES)],
                        axis=0).astype(np.int32)
    pi = np.sum(np.stack([results[c]["pio"][0] for c in range(NCORES)]),
                axis=0, dtype=np.float32) / np.float32(N)
    counts = np.bincount(i8.reshape(-1), minlength=E).astype(np.float32)
    fi = counts / np.float32(N * K) * np.float32(E)
    aux = np.float32(np.sum(pi * fi, dtype=np.float32) * AUX_ALPHA)
    return w8, i8, aux


def kernel(x, weight):
    res = _run_device(x, weight, trace=False)
    return _assemble(res.results)


# revision 25
# speedup vs baseline: 1.0618x; 1.0275x over previous
"""MoE gate (softmax -> top-8 -> renorm + aux-loss stats) on 8 trn2 cores.

Sharding: data-parallel over the flattened token dim (2048 tokens/core).
Each core receives its x-shard pre-transposed to [D, T] so the contraction
dim (D) lies on SBUF partitions, plus the replicated gate weight transposed
to [D, E].

Device kernel (per core):
  - 16 d-chunk matmuls accumulate logits [128 tok, 64 exp] in PSUM
  - ScalarE: exp + row-sum (accum_out) in one instruction
  - VectorE: reciprocal, max (top-8 desc), max_index (exact argmax-8),
    top-8 renorm (sum/recip/mul)
  - pi partial: PE matmul with the reciprocal column stationary:
    [1,64] += r.T @ exp  == column-sums of softmax scores
Host: gathers per-core outputs, computes fi from an index histogram and
the scalar aux loss (64-element math).
"""

import os
import sys

import numpy as np

for _p in ("/opt/trn_rl_repo", "/root/.axon_site/_ro/trn_rl_repo"):
    if os.path.isdir(_p) and _p not in sys.path:
        sys.path.insert(0, _p)

import concourse.bacc as bacc
import concourse.mybir as mybir
import concourse.tile as tile
from concourse import masks
from concourse.bass_utils import run_bass_kernel_spmd

B, S, D = 4, 4096, 2048
E, K = 64, 8
NCORES = 8
N = B * S
T = N // NCORES            # 2048 tokens per core
TB = 512                   # tokens per DMA block
NB = T // TB               # 4 blocks
TT = TB // 128             # 4 token-tiles per block
DC = D // 128              # 16 contraction chunks
AUX_ALPHA = np.float32(0.01)

_cache = {}


def _build_nc():
    f32 = mybir.dt.float32
    u32 = mybir.dt.uint32
    nc = bacc.Bacc(None)
    xt = nc.dram_tensor("xt", [D, T], f32, kind="ExternalInput")
    # weight pre-arranged on host to the SBUF layout: [128, DC*E], where
    # column block c holds wT[c*128:(c+1)*128, :] for partition rows 0..127
    wt = nc.dram_tensor("wt", [128, DC * E], f32, kind="ExternalInput")
    w8o = nc.dram_tensor("w8o", [T, K], f32, kind="ExternalOutput")
    i8o = nc.dram_tensor("i8o", [T, K], u32, kind="ExternalOutput")
    pio = nc.dram_tensor("pio", [1, E], f32, kind="ExternalOutput")

    with tile.TileContext(nc) as tc:
        with (
            tc.tile_pool(name="wpool", bufs=1) as wpool,
            tc.tile_pool(name="xpool", bufs=32) as xpool,
            tc.tile_pool(name="lpool", bufs=4) as lpool,
            tc.tile_pool(name="epool", bufs=4) as epool,
            tc.tile_pool(name="opool", bufs=3) as opool,
            tc.tile_pool(name="small", bufs=4) as spool,
            tc.tile_pool(name="acc", bufs=2) as apool,
            tc.tile_pool(name="lgpsum", bufs=4, space="PSUM") as lgpool,
            tc.tile_pool(name="trpsum", bufs=3, space="PSUM") as trpool,
            tc.tile_pool(name="pipsum", bufs=1, space="PSUM") as pipool,
        ):
            # Replicated gate weight, already in SBUF layout on the host:
            # single fully-contiguous DMA, first in the sync-ring queue so it
            # completes before the first matmul needs it.
            wsb = wpool.tile([128, DC * E], f32)
            nc.sync.dma_start(wsb[:], wt[:])
            ident = wpool.tile([64, 64], f32)
            masks.make_identity(nc, ident[:])
            ones = wpool.tile([128, 1], f32)
            nc.vector.memset(ones[:], 1.0)

            # pi accumulates in PSUM across blocks: pp += ones.T @ sacc_b
            pp = pipool.tile([1, E], f32)

            def make_tile_thunks(b, lgs):
                # 4 deferred per-tile epilogues: PE transpose back to
                # [tok, exp], exp+rowsum and score scaling on ACT, top-8 +
                # renorm on DVE. The last tile also folds the block's pi
                # matmul and the batched output DMAs.
                wstage = opool.tile([128, TT * K], f32, tag="wstage")
                istage = opool.tile([128, TT * K], u32, tag="istage")
                sacc = apool.tile([128, E], f32, tag="sacc")

                def tile_thunk(t):
                    tr = trpool.tile([128, E], f32, tag="tr")
                    nc.tensor.transpose(
                        tr[:], lgs[:, t * 128:(t + 1) * 128], ident[:])
                    ex = epool.tile([128, E], f32, tag="ex")
                    esum = spool.tile([128, 1], f32, tag="esum")
                    nc.scalar.activation(
                        ex[:], tr[:], mybir.ActivationFunctionType.Exp,
                        accum_out=esum[:])
                    r = spool.tile([128, 1], f32, tag="r")
                    nc.vector.reciprocal(r[:], esum[:])
                    e8 = spool.tile([128, 8], f32, tag="e8")
                    nc.vector.max(e8[:], ex[:])
                    nc.vector.max_index(istage[:, t * K:(t + 1) * K], e8[:], ex[:])
                    s8 = spool.tile([128, 1], f32, tag="s8")
                    nc.vector.reduce_sum(s8[:], e8[:], axis=mybir.AxisListType.X)
                    rs = spool.tile([128, 1], f32, tag="rs")
                    nc.vector.reciprocal(rs[:], s8[:])
                    nc.vector.tensor_scalar_mul(
                        wstage[:, t * K:(t + 1) * K], e8[:], rs[:])
                    # scores tile s = ex * r on ACT (Copy with per-row scale)
                    s = epool.tile([128, E], f32, tag="s")
                    nc.scalar.activation(
                        s[:], ex[:], mybir.ActivationFunctionType.Copy,
                        scale=r[:])
                    if t == 0:
                        nc.vector.tensor_copy(sacc[:], s[:])
                    else:
                        nc.vector.tensor_add(sacc[:], sacc[:], s[:])
                    if t == TT - 1:
                        nc.tensor.matmul(pp[:], ones[:], sacc[:],
                                         start=(b == 0), stop=(b == NB - 1),
                                         skip_group_check=True)
                        nc.scalar.dma_start(
                            w8o[b * TB:(b + 1) * TB, :].rearrange(
                                "(t p) k -> p t k", p=128),
                            wstage[:].rearrange("p (t k) -> p t k", t=TT))
                        nc.scalar.dma_start(
                            i8o[b * TB:(b + 1) * TB, :].rearrange(
                                "(t p) k -> p t k", p=128),
                            istage[:].rearrange("p (t k) -> p t k", t=TT))

                return [lambda t=t: tile_thunk(t) for t in range(TT)]

            # Per DMA pair (2 blocks, [128, 2*TB] chunk tiles): the two
            # blocks' matmuls are interleaved per chunk-pair so PE trails the
            # DMA stream by one chunk, and the previous pair's epilogue tiles
            # are woven between chunk-pairs to fill PE gaps.
            pend = []
            for p in range(NB // 2):
                b0, b1 = 2 * p, 2 * p + 1
                xcs = []
                for c in range(DC):
                    xc = xpool.tile([128, 2 * TB], f32, tag="xc")
                    nc.sync.dma_start(
                        xc[:],
                        xt[c * 128:(c + 1) * 128, b0 * TB:(b1 + 1) * TB])
                    xcs.append(xc)
                lgA = lgpool.tile([128, TB], f32, tag="lg")
                lgB = lgpool.tile([128, TB], f32, tag="lg")
                for ci in range(0, DC, 2):
                    for lg, half in ((lgA, 0), (lgB, TB)):
                        nc.tensor.matmul(
                            lg[0:64, :],
                            wsb[:, ci * E:(ci + 1) * E],
                            xcs[ci][:, half:half + TB],
                            start=(ci == 0),
                            stop=(ci == DC - 2),
                            tile_position=(0, 0),
                            skip_group_check=True,
                        )
                        nc.tensor.matmul(
                            lg[64:128, :],
                            wsb[:, (ci + 1) * E:(ci + 2) * E],
                            xcs[ci + 1][:, half:half + TB],
                            start=(ci == 0),
                            stop=(ci == DC - 2),
                            tile_position=(0, 64),
                            skip_group_check=True,
                        )
                    if ci >= 2 and pend:
                        pend.pop(0)()
                newpend = []
                for b, lg in ((b0, lgA), (b1, lgB)):
                    lgt = lpool.tile([64, TB], f32, tag="lgt")
                    nc.scalar.copy(lgt[:], lg[64:128, :])
                    lgs = lpool.tile([64, TB], f32, tag="lgs")
                    nc.vector.tensor_add(lgs[:], lg[0:64, :], lgt[:])
                    newpend += make_tile_thunks(b, lgs)
                while pend:
                    pend.pop(0)()
                pend = newpend
            while pend:
                pend.pop(0)()
            pi_sb = spool.tile([1, E], f32, tag="pisb")
            nc.vector.tensor_copy(pi_sb[:], pp[:])
            nc.sync.dma_start(pio[:], pi_sb[:])
    nc.compile()
    return nc


def _get_nc():
    if "nc" not in _cache:
        _cache["nc"] = _build_nc()
    return _cache["nc"]


def _make_in_maps(x, weight):
    xf = np.ascontiguousarray(
        np.asarray(x, dtype=np.float32).reshape(N, D))
    wT = np.asarray(weight, dtype=np.float32).T          # [D, E]
    # SBUF layout: [128, DC*E], column block c = wT[c*128:(c+1)*128, :]
    wsb = np.ascontiguousarray(
        wT.reshape(DC, 128, E).transpose(1, 0, 2).reshape(128, DC * E))
    in_maps = []
    for cid in range(NCORES):
        xTs = np.ascontiguousarray(xf[cid * T:(cid + 1) * T].T)
        in_maps.append({"xt": xTs, "wt": wsb})
    return in_maps


def _run_device(x, weight, trace=False, **kw):
    nc = _get_nc()
    in_maps = _make_in_maps(x, weight)
    return run_bass_kernel_spmd(nc, in_maps, list(range(NCORES)), trace=trace, **kw)


def _assemble(results):
    w8 = np.concatenate([results[c]["w8o"] for c in range(NCORES)], axis=0)
    i8 = np.concatenate([results[c]["i8o"] for c in range(NCORES)],
                        axis=0).astype(np.int32)
    pi = np.sum(np.stack([results[c]["pio"][0] for c in range(NCORES)]),
                axis=0, dtype=np.float32) / np.float32(N)
    counts = np.bincount(i8.reshape(-1), minlength=E).astype(np.float32)
    fi = counts / np.float32(N * K) * np.float32(E)
    aux = np.float32(np.sum(pi * fi, dtype=np.float32) * AUX_ALPHA)
    return w8, i8, aux


def kernel(x, weight):
    res = _run_device(x, weight, trace=False)
    return _assemble(res.results)
